# revision 41
# baseline (speedup 1.0000x reference)
"""DeltaNet model kernel for 8 Trainium2 NeuronCores.

Sharding: data-parallel over batch (2) x tensor-parallel over vocab (4) for
the LM head; each core runs the full 2-layer backbone for its batch element
and computes logits for its 8000-vocab shard.  No inter-core communication.

The delta-rule scan is evaluated in closed "chunked attention" form
(chunk=128): per-chunk inverse of (I + strict_tril(beta * K K^T)) via exact
nilpotent squaring, then all cross-chunk interactions as dense matmuls.

Numerics: fp16 weights/activations with fp32 PSUM accumulation everywhere;
U recursion state kept in f32r.  Logits are stored fp16 on device and
widened to f32 on the host.
"""

import sys

for _p in ("/opt/trn_rl_repo",):
    if _p not in sys.path:
        sys.path.insert(0, _p)

import numpy as np
import ml_dtypes

import concourse.bass as bass
import concourse.mybir as mybir
from concourse import bacc
from concourse.bass_utils import run_bass_kernel_spmd
from concourse.tile import TileContext
from concourse.masks import make_identity, make_upper_triangular

P = 128
D = 1024
S = 1024
V = 32000
L = 2
NCH = 8           # token chunks of 128
DSUB = 8          # D / P
VS = V // 4       # vocab shard = 8000
VTS = 63          # padded v-tiles (63*128 = 8064)
VSP = VTS * P

F32 = mybir.dt.float32
F32R = mybir.dt.float32r
F16 = mybir.dt.float16
I32 = mybir.dt.int32
AF = mybir.ActivationFunctionType
ALU = mybir.AluOpType
F16_NP = np.float16

EPS_L2 = 1e-6
EPS_RMS = 1e-5
EPS_LN = 1e-5


def ts(i, n):
    return slice(i * n, (i + 1) * n)


def build_program():
    nc = bacc.Bacc("TRN2", target_bir_lowering=False, debug=False, num_devices=8)

    tok_d = nc.dram_tensor("tokens", (P, NCH), I32, kind="ExternalInput").ap()
    emb_d = nc.dram_tensor("emb", (V, D), F16, kind="ExternalInput").ap()
    wq_d = nc.dram_tensor("wq", (L, P, DSUB, D), F16, kind="ExternalInput").ap()
    wk_d = nc.dram_tensor("wk", (L, P, DSUB, D), F16, kind="ExternalInput").ap()
    wv_d = nc.dram_tensor("wv", (L, P, DSUB, D), F16, kind="ExternalInput").ap()
    wb_d = nc.dram_tensor("wb", (L, P, DSUB, 2), F16, kind="ExternalInput").ap()
    wo_d = nc.dram_tensor("wo", (L, P, DSUB, D), F16, kind="ExternalInput").ap()
    lng_d = nc.dram_tensor("lng", (P, DSUB), F32, kind="ExternalInput").ap()
    lnb_d = nc.dram_tensor("lnb", (P, DSUB), F32, kind="ExternalInput").ap()
    hw_d = nc.dram_tensor("hw", (VTS // 3, P, 3, DSUB, P), F16,
                      kind="ExternalInput").ap()
    out_d = nc.dram_tensor("logits_t", (VSP, S), F16, kind="ExternalOutput").ap()

    with TileContext(nc) as tc:
        _build(nc, tc, tok_d, emb_d, wq_d, wk_d, wv_d, wb_d, wo_d,
               lng_d, lnb_d, hw_d, out_d)
    nc.compile()
    return nc


def _build(nc, tc, tok_d, emb_d, wq_d, wk_d, wv_d, wb_d, wo_d,
           lng_d, lnb_d, hw_d, out_d):
    from contextlib import ExitStack
    ctx = ExitStack()
    pool = ctx.enter_context(tc.tile_pool(name="main", bufs=1))
    ring = ctx.enter_context(tc.tile_pool(name="ring", bufs=2))
    scr = ctx.enter_context(tc.tile_pool(name="scr", bufs=3))
    wpool = ctx.enter_context(tc.tile_pool(name="w", bufs=2))
    hppool = ctx.enter_context(tc.tile_pool(name="hp", bufs=8))
    zpool = ctx.enter_context(tc.tile_pool(name="z", bufs=16))
    gpool = ctx.enter_context(tc.tile_pool(name="g", bufs=8))
    sm2 = ctx.enter_context(tc.tile_pool(name="sm2", bufs=4))
    sm4 = ctx.enter_context(tc.tile_pool(name="sm4", bufs=4))
    sm8 = ctx.enter_context(tc.tile_pool(name="sm8", bufs=8))
    rows = ctx.enter_context(tc.tile_pool(name="rows", bufs=3))
    outp = ctx.enter_context(tc.tile_pool(name="outp", bufs=2))
    hwp = ctx.enter_context(tc.tile_pool(name="hwp", bufs=2))
    pa = ctx.enter_context(tc.tile_pool(name="pa", bufs=4, space="PSUM"))
    pb = ctx.enter_context(tc.tile_pool(name="pb", bufs=4, space="PSUM"))

    # ---- constants ----
    ident_f = pool.tile([P, P], F32, tag="identf")
    make_identity(nc, ident_f[:])
    ident_b = pool.tile([P, P], F16, tag="identb")
    nc.vector.tensor_copy(ident_b[:], ident_f[:])
    mask_ui = pool.tile([P, P], F32, tag="mui")      # 1 where i <= t (upper incl)
    make_upper_triangular(nc, mask_ui[:], val=1.0, diag=True)
    mask_su = pool.tile([P, P], F32, tag="msu")      # 1 where i < t (strict upper)
    make_upper_triangular(nc, mask_su[:], val=1.0, diag=False)
    mask_sun = pool.tile([P, P], F32, tag="msun")    # -1 where i < t
    make_upper_triangular(nc, mask_sun[:], val=-1.0, diag=False)
    ones_f = pool.tile([P, 1], F32, tag="onesf")
    nc.gpsimd.memset(ones_f[:], 1.0)
    ones_r = pool.tile([P, 1], F16, tag="onesr")     # ones column (f16)
    nc.vector.tensor_copy(ones_r[:], ones_f[:])
    ones_row_f = pool.tile([1, P], F32, tag="onesrowf")
    nc.gpsimd.memset(ones_row_f[:], 1.0)
    ones_row = pool.tile([1, P], F32R, tag="onesrow")  # ones row for bcast
    nc.vector.tensor_copy(ones_row[:], ones_row_f[:])
    eps6_t = pool.tile([1, 1], F32, tag="eps6")   # 1e-6 (l2norm)
    nc.gpsimd.memset(eps6_t[:], EPS_L2)
    eps5_t = pool.tile([1, 1], F32, tag="eps5")   # 1e-5 (rms / ln)
    nc.gpsimd.memset(eps5_t[:], EPS_RMS)
    lng_sb = pool.tile([P, DSUB], F32, tag="lng")
    nc.sync.dma_start(lng_sb[:], lng_d[:])
    lnb_sb = pool.tile([P, DSUB], F32, tag="lnb")
    nc.sync.dma_start(lnb_sb[:], lnb_d[:])

    # ---- residual stream (feature-major): xfm[p, do, s] = x[s, do*128+p] ----
    xfm = pool.tile([P, DSUB, S], F16, tag="xfm")

    # ---- embedding gather + transpose to feature-major ----
    tok_sb = pool.tile([P, NCH], I32, tag="tok")
    nc.sync.dma_start(tok_sb[:], tok_d[:])
    for st in range(NCH):
        xg = ring.tile([P, D], F16, tag="vc")
        nc.gpsimd.indirect_dma_start(
            out=xg[:], out_offset=None, in_=emb_d[:],
            in_offset=bass.IndirectOffsetOnAxis(ap=tok_sb[:, st:st + 1], axis=0))
        for do in range(DSUB):
            pt = pb.tile([P, 256], F16, tag="pb")
            nc.tensor.transpose(pt[:, :P], xg[:, ts(do, P)], ident_b[:])
            nc.vector.tensor_copy(xfm[:, do, ts(st, P)], pt[:, :P])

    kfm = pool.tile([P, DSUB, S], F16, tag="kfm")
    qfm = pool.tile([P, DSUB, S], F16, tag="qfm")
    v_tm = pool.tile([P, NCH, D], F16, tag="vtm")
    u_tm = pool.tile([P, NCH, D], F16, tag="u")
    beta_tm = pool.tile([P, NCH], F32, tag="btm")
    beta_fm = pool.tile([1, S], F32R, tag="bfm")

    for l in range(L):
        # ==== layer weights, each loaded once ====
        wk_sb = wpool.tile([P, DSUB, D], F16, tag="w", name=f"wk{l}")
        nc.sync.dma_start(wk_sb[:], wk_d[l])
        wq_sb = wpool.tile([P, DSUB, D], F16, tag="w", name=f"wq{l}")
        nc.sync.dma_start(wq_sb[:], wq_d[l])
        wv_sb = wpool.tile([P, DSUB, D], F16, tag="w", name=f"wv{l}")
        nc.sync.dma_start(wv_sb[:], wv_d[l])
        wo_sb = wpool.tile([P, DSUB, D], F16, tag="w", name=f"wo{l}")
        nc.sync.dma_start(wo_sb[:], wo_d[l])
        wbt = wpool.tile([P, DSUB, 2], F16, tag="wb")
        nc.sync.dma_start(wbt[:], wb_d[l])

        # ==== k and q projections (feature-major) + silu ====
        for dkt in range(DSUB):  # 128-wide chunks of the output dim
            for sh in range(2):  # 512-wide s halves
                ps = pa.tile([P, 512], F32, tag="pa")
                for ko in range(DSUB):
                    nc.tensor.matmul(ps[:], wk_sb[:, ko, ts(dkt, P)],
                                     xfm[:, ko, ts(sh, 512)],
                                     start=(ko == 0), stop=(ko == DSUB - 1))
                sc = scr.tile([P, 512], F32, tag="scr")
                nc.scalar.activation(sc[:], ps[:], AF.Sigmoid)
                nc.vector.tensor_tensor(kfm[:, dkt, ts(sh, 512)], ps[:], sc[:],
                                        ALU.mult)
                ps2 = pa.tile([P, 512], F32, tag="pa")
                for ko in range(DSUB):
                    nc.tensor.matmul(ps2[:], wq_sb[:, ko, ts(dkt, P)],
                                     xfm[:, ko, ts(sh, 512)],
                                     start=(ko == 0), stop=(ko == DSUB - 1))
                sc2 = scr.tile([P, 512], F32, tag="scr")
                nc.scalar.activation(sc2[:], ps2[:], AF.Sigmoid)
                nc.vector.tensor_tensor(qfm[:, dkt, ts(sh, 512)], ps2[:], sc2[:],
                                        ALU.mult)

        # l2-norm row sums: sumsq over dk (partition dim) via ones-matmul
        ssk_ps = [pa.tile([P, 512], F32, tag="pa", name=f"ssk{l}_{i}")
                  for i in range(2)]
        ssq_ps = [pa.tile([P, 512], F32, tag="pa", name=f"ssq{l}_{i}")
                  for i in range(2)]
        for dkt in range(DSUB):
            for sh in range(2):
                sq = scr.tile([P, 512], F16, tag="scrb")
                nc.vector.tensor_tensor(sq[:], kfm[:, dkt, ts(sh, 512)],
                                        kfm[:, dkt, ts(sh, 512)], ALU.mult)
                nc.tensor.matmul(ssk_ps[sh][:1, :], ones_r[:], sq[:],
                                 start=(dkt == 0), stop=(dkt == DSUB - 1))
                sq2 = scr.tile([P, 512], F16, tag="scrb")
                nc.vector.tensor_tensor(sq2[:], qfm[:, dkt, ts(sh, 512)],
                                        qfm[:, dkt, ts(sh, 512)], ALU.mult)
                nc.tensor.matmul(ssq_ps[sh][:1, :], ones_r[:], sq2[:],
                                 start=(dkt == 0), stop=(dkt == DSUB - 1))
        # rk rows (1/||k||); rq rows for q are folded into the output scale
        rk_row = rows.tile([1, S], F32R, tag="rkrow", bufs=1)
        rq_row = rows.tile([1, S], F32R, tag="rqrow", bufs=1)
        for sh in range(2):
            s_ = rows.tile([1, 512], F32, tag="srow")
            nc.scalar.activation(s_[:], ssk_ps[sh][:1, :], AF.Sqrt,
                                 bias=eps6_t[:])
            with nc.allow_low_precision(reason="f32r == f32 bits"):
                nc.vector.reciprocal(rk_row[:, ts(sh, 512)], s_[:])
            s2_ = rows.tile([1, 512], F32, tag="srow")
            nc.scalar.activation(s2_[:], ssq_ps[sh][:1, :], AF.Sqrt,
                                 bias=eps6_t[:])
            with nc.allow_low_precision(reason="f32r == f32 bits"):
                nc.vector.reciprocal(rq_row[:, ts(sh, 512)], s2_[:])
        for sh in range(2):
            psb = pa.tile([P, 512], F32, tag="pa")
            nc.tensor.matmul(psb[:], ones_row[:], rk_row[:, ts(sh, 512)],
                             start=True, stop=True)
            rk_bc = ring.tile([P, 512], F32, tag="bc2")
            nc.vector.tensor_copy(rk_bc[:], psb[:])
            for dkt in range(DSUB):
                nc.vector.tensor_tensor(kfm[:, dkt, ts(sh, 512)],
                                        kfm[:, dkt, ts(sh, 512)], rk_bc[:],
                                        ALU.mult)

        # ==== beta (token-major and feature-major) ====
        for st in range(NCH):
            psb = pb.tile([P, 256], F32, tag="pb")
            for ko in range(DSUB):
                nc.tensor.matmul(psb[:, :2], xfm[:, ko, ts(st, P)], wbt[:, ko, :],
                                 start=(ko == 0), stop=(ko == DSUB - 1))
            nc.scalar.activation(beta_tm[:, st:st + 1], psb[:, :1], AF.Sigmoid)
        for sh in range(2):
            psb = pa.tile([P, 512], F32, tag="pa")
            for ko in range(DSUB):
                nc.tensor.matmul(psb[:2, :], wbt[:, ko, :], xfm[:, ko, ts(sh, 512)],
                                 start=(ko == 0), stop=(ko == DSUB - 1))
            nc.scalar.activation(beta_fm[:, ts(sh, 512)], psb[:1, :], AF.Sigmoid)

        # ==== v = silu(x Wv), token-major, SBUF-resident ====
        for st in range(NCH):
            for half in range(2):
                ps = pa.tile([P, 512], F32, tag="pa")
                for ko in range(DSUB):
                    nc.tensor.matmul(ps[:], xfm[:, ko, ts(st, P)],
                                     wv_sb[:, ko, ts(half, 512)],
                                     start=(ko == 0), stop=(ko == DSUB - 1))
                scv = scr.tile([P, 512], F32, tag="scr")
                nc.scalar.activation(scv[:], ps[:], AF.Sigmoid)
                nc.vector.tensor_tensor(v_tm[:, st, ts(half, 512)], ps[:],
                                        scv[:], ALU.mult)

        # ==== chunk inverses: P_c = diag(beta) T_c^T, T = (I+A)^-1 ====
        # T^T = (I+Y)^-1 = prod_k (I + Z^(2^k)) with Y = N^T, Z = -Y; the 8
        # per-chunk chains are emitted step-interleaved so the in-order
        # engine queues pipeline them against each other.
        Ptiles = []
        jccs, zs, zts, aas = [], [], [], []
        for c in range(NCH):
            jps = pb.tile([P, 256], F32, tag="pb")
            for ko in range(DSUB):
                nc.tensor.matmul(jps[:, :P], kfm[:, ko, ts(c, P)],
                                 kfm[:, ko, ts(c, P)],
                                 start=(ko == 0), stop=(ko == DSUB - 1))
            jcc = sm8.tile([P, P], F16, tag="jcc", bufs=8)
            nc.scalar.copy(jcc[:], jps[:, :P])
            jccs.append(jcc)
        for c in range(NCH):
            # Z = -N^T = strict_triu(-beta_col * J); Z^T = -strict_tril(b_r*J)
            tmp = scr.tile([P, 512], F32, tag="scr")
            nc.vector.tensor_scalar_mul(tmp[:, :P], jccs[c][:],
                                        beta_tm[:, c:c + 1])
            tmp2 = scr.tile([P, 512], F32, tag="scr")
            nc.vector.tensor_tensor(tmp2[:, :P], tmp[:, :P], mask_ui[:],
                                    ALU.mult)
            zt0 = zpool.tile([P, P], F16, tag="zt", bufs=16)
            nc.vector.tensor_tensor(zt0[:], tmp2[:, :P], tmp[:, :P],
                                    ALU.subtract)
            bps = pb.tile([P, 256], F32, tag="pb")
            nc.tensor.matmul(bps[:, :P], ones_row[:], beta_fm[:, ts(c, P)],
                             start=True, stop=True)
            mbn = sm8.tile([P, P], F16, tag="mbn", bufs=8)
            nc.vector.tensor_tensor(mbn[:], bps[:, :P], mask_sun[:], ALU.mult)
            z0 = zpool.tile([P, P], F16, tag="z", bufs=16)
            nc.vector.tensor_tensor(z0[:], mbn[:], jccs[c][:], ALU.mult)
            a0 = zpool.tile([P, P], F16, tag="a", bufs=16)
            nc.vector.tensor_tensor(a0[:], ident_f[:], z0[:], ALU.add)
            zs.append(z0)
            zts.append(zt0)
            aas.append(a0)
        for kk in range(1, 7):
            z_olds = list(zs)
            for c in range(NCH):  # Z^(2^k), wave A (zt_6 alone suffices at kk=6)
                if kk == 6:
                    break
                psq = pb.tile([P, 256], F32, tag="pb")
                nc.tensor.matmul(psq[:, :P], zts[c][:], z_olds[c][:],
                                 start=True, stop=True)
                z_new = zpool.tile([P, P], F16, tag="z", bufs=16)
                nc.vector.tensor_copy(z_new[:], psq[:, :P])
                zs[c] = z_new
            for c in range(NCH):  # (Z^(2^k))^T, wave B
                psqt = pb.tile([P, 256], F32, tag="pb")
                nc.tensor.matmul(psqt[:, :P], z_olds[c][:], zts[c][:],
                                 start=True, stop=True)
                zt_new = zpool.tile([P, P], F16, tag="zt", bufs=16)
                nc.scalar.copy(zt_new[:], psqt[:, :P])
                zts[c] = zt_new
            for c in range(NCH):  # A = (I + Z^(2^k)) A, wave C
                psm = pb.tile([P, 256], F32, tag="pb")
                nc.tensor.matmul(psm[:, :P], zts[c][:], aas[c][:],
                                 start=True, stop=False)
                nc.tensor.matmul(psm[:, :P], ident_b[:], aas[c][:],
                                 start=False, stop=True)
                if kk < 6:
                    a_new = zpool.tile([P, P], F16, tag="a", bufs=16)
                    nc.vector.tensor_copy(a_new[:], psm[:, :P])
                    aas[c] = a_new
                else:
                    p_c = sm8.tile([P, P], F16, tag="pc")
                    nc.vector.tensor_scalar_mul(p_c[:], psm[:, :P],
                                                beta_tm[:, c:c + 1])
                    Ptiles.append(p_c)

        # ==== scan ====
        for cp in range(4):
            c0, c1 = 2 * cp, 2 * cp + 1
            for c in (c0, c1):
                # --- negated J pair tiles for j < c ---
                jsbs = []
                for jp in range((c + 1) // 2):
                    jps = pb.tile([P, 256], F32, tag="pb")
                    for ko in range(DSUB):
                        nc.tensor.matmul(jps[:], kfm[:, ko, ts(c, P)],
                                         kfm[:, ko, ts(jp, 256)],
                                         start=(ko == 0), stop=(ko == DSUB - 1))
                    jsb = sm4.tile([P, 256], F16, tag="jsb")
                    nc.vector.tensor_scalar_mul(jsb[:], jps[:], -1.0)
                    jsbs.append(jsb)
                # --- -G_cj tiles, shared across both halves ---
                js = list(range(c))
                gnegs = []
                for j in js:
                    gps = pb.tile([P, 256], F32, tag="pb")
                    nc.tensor.matmul(gps[:, :P], jsbs[j // 2][:, ts(j % 2, P)],
                                     Ptiles[c][:], start=True, stop=True)
                    gneg = gpool.tile([P, P], F16, tag="gneg")
                    nc.vector.tensor_copy(gneg[:], gps[:, :P])
                    gnegs.append(gneg)
                # --- U_c = (T B) V_c - sum_j G_cj U_j ---
                for half in range(2):
                    psu = pa.tile([P, 512], F32, tag="pa")
                    nc.tensor.matmul(psu[:], Ptiles[c][:],
                                     v_tm[:, c, ts(half, 512)],
                                     start=True, stop=(len(js) == 0))
                    for gi, j in enumerate(js):
                        nc.tensor.matmul(psu[:], gnegs[gi][:],
                                         u_tm[:, j, ts(half, 512)],
                                         start=False, stop=(gi == len(js) - 1))
                    nc.vector.tensor_copy(u_tm[:, c, ts(half, 512)], psu[:])

            # --- H^T pair tiles for this cp ---
            hps = []
            for j in range(c1 + 1):
                php = pb.tile([P, 256], F32, tag="pb")
                for ko in range(DSUB):
                    nc.tensor.matmul(php[:], kfm[:, ko, ts(j, P)],
                                     qfm[:, ko, ts(cp, 256)],
                                     start=(ko == 0), stop=(ko == DSUB - 1))
                hp = hppool.tile([P, 256], F16, tag="hp")
                if j == c0:
                    nc.vector.tensor_tensor(hp[:, :P], php[:, :P], mask_ui[:],
                                            ALU.mult)
                    nc.vector.tensor_copy(hp[:, P:], php[:, P:])
                elif j == c1:
                    nc.vector.tensor_tensor(hp[:, P:], php[:, P:], mask_ui[:],
                                            ALU.mult)
                else:
                    nc.vector.tensor_copy(hp[:], php[:])
                hps.append(hp)
            # --- O feature-major, accumulate over j per e-tile ---
            on_c = ring.tile([P, DSUB, 256], F16, tag="on", bufs=1)
            sso_ps = pb.tile([P, 256], F32, tag="pb")
            for wave in range(2):
                opss = []
                for ei in range(4):
                    et = wave * 4 + ei
                    pso = pb.tile([P, 256], F32, tag="pb")
                    for j in range(c1 + 1):
                        if j == c1:
                            nc.tensor.matmul(pso[:, P:], u_tm[:, j, ts(et, P)],
                                             hps[j][:, P:], start=False, stop=True)
                        else:
                            nc.tensor.matmul(pso[:], u_tm[:, j, ts(et, P)],
                                             hps[j][:], start=(j == 0), stop=False)
                    opss.append((et, pso))
                for et, pso in opss:
                    nc.vector.tensor_copy(on_c[:, et, :], pso[:])
                    sq = scr.tile([P, 512], F16, tag="scrb")
                    nc.vector.tensor_tensor(sq[:, :256], on_c[:, et, :],
                                            on_c[:, et, :], ALU.mult)
                    nc.tensor.matmul(sso_ps[:1, :], ones_r[:], sq[:, :256],
                                     start=(et == 0), stop=(et == DSUB - 1))
            # combined scale row: a = rq / sqrt(rq^2 * sso / D + eps_rms)
            rq2 = rows.tile([1, 512], F32, tag="srow")
            nc.vector.tensor_tensor(rq2[:, :256], rq_row[:, ts(cp, 256)],
                                    rq_row[:, ts(cp, 256)], ALU.mult)
            nc.vector.tensor_scalar_mul(rq2[:, :256], rq2[:, :256], 1.0 / D)
            ssos = rows.tile([1, 512], F32, tag="srow")
            nc.vector.tensor_tensor(ssos[:, :256], sso_ps[:1, :], rq2[:, :256],
                                    ALU.mult)
            nc.scalar.activation(ssos[:, :256], ssos[:, :256], AF.Sqrt,
                                 bias=eps5_t[:])
            row_a = rows.tile([1, 512], F32R, tag="srowr", bufs=2)
            with nc.allow_low_precision(reason="f32r == f32 bits"):
                nc.vector.reciprocal(row_a[:, :256], ssos[:, :256])
            nc.vector.tensor_tensor(row_a[:, :256], row_a[:, :256],
                                    rq_row[:, ts(cp, 256)], ALU.mult)
            psb = pb.tile([P, 256], F32, tag="pb")
            nc.tensor.matmul(psb[:], ones_row[:], row_a[:, :256],
                             start=True, stop=True)
            a_bc = sm2.tile([P, 256], F32, tag="abc")
            nc.vector.tensor_copy(a_bc[:], psb[:])

            # --- x_next columns for this cp (per-token scale fused in) ---
            for do in range(DSUB):
                psx = pb.tile([P, 256], F32, tag="pb")
                for ko in range(DSUB):
                    nc.tensor.matmul(psx[:], wo_sb[:, ko, ts(do, P)],
                                     on_c[:, ko, :],
                                     start=(ko == 0), stop=(ko == DSUB - 1))
                nc.vector.tensor_tensor(xfm[:, do, ts(cp, 256)], psx[:],
                                        a_bc[:], ALU.mult)

    # ==== final layernorm, one token-half ====
    def emit_ln(sh):
        sum_ps = pa.tile([P, 512], F32, tag="pa")
        ssq_ps = pa.tile([P, 512], F32, tag="pa")
        for do in range(DSUB):
            nc.tensor.matmul(sum_ps[:1, :], ones_r[:], xfm[:, do, ts(sh, 512)],
                             start=(do == 0), stop=(do == DSUB - 1))
            sq = scr.tile([P, 512], F16, tag="scrb")
            nc.vector.tensor_tensor(sq[:], xfm[:, do, ts(sh, 512)],
                                    xfm[:, do, ts(sh, 512)], ALU.mult)
            nc.tensor.matmul(ssq_ps[:1, :], ones_r[:], sq[:],
                             start=(do == 0), stop=(do == DSUB - 1))
        mu = rows.tile([1, 512], F32, tag="srow")
        nc.vector.tensor_scalar_mul(mu[:], sum_ps[:1, :], 1.0 / D)
        m2_ = rows.tile([1, 512], F32, tag="srow")
        nc.vector.tensor_scalar_mul(m2_[:], ssq_ps[:1, :], 1.0 / D)
        mu2 = rows.tile([1, 512], F32, tag="srow")
        nc.vector.tensor_tensor(mu2[:], mu[:], mu[:], ALU.mult)
        nc.vector.tensor_tensor(m2_[:], m2_[:], mu2[:], ALU.subtract)
        nc.scalar.activation(mu2[:], m2_[:], AF.Sqrt, bias=eps5_t[:])
        row_a = rows.tile([1, 512], F32R, tag="srowr", bufs=2)
        with nc.allow_low_precision(reason="f32r == f32 bits"):
            nc.vector.reciprocal(row_a[:], mu2[:])
        nc.vector.tensor_scalar_mul(mu[:], mu[:], -1.0)
        row_b = rows.tile([1, 512], F32R, tag="srowr", bufs=2)
        nc.vector.tensor_tensor(row_b[:], mu[:], row_a[:], ALU.mult)
        psb = pa.tile([P, 512], F32, tag="pa")
        nc.tensor.matmul(psb[:], ones_row[:], row_a[:], start=True, stop=True)
        a_bc = ring.tile([P, 512], F32, tag="bc2")
        nc.vector.tensor_copy(a_bc[:], psb[:])
        psb = pa.tile([P, 512], F32, tag="pa")
        nc.tensor.matmul(psb[:], ones_row[:], row_b[:], start=True, stop=True)
        b_bc = ring.tile([P, 512], F32, tag="bc2")
        nc.vector.tensor_copy(b_bc[:], psb[:])
        for do in range(DSUB):
            t1 = scr.tile([P, 512], F32, tag="scr")
            nc.vector.tensor_tensor(t1[:], xfm[:, do, ts(sh, 512)], a_bc[:],
                                    ALU.mult)
            nc.vector.tensor_tensor(t1[:], t1[:], b_bc[:], ALU.add)
            nc.vector.tensor_scalar(t1[:], t1[:], lng_sb[:, do:do + 1],
                                    lnb_sb[:, do:do + 1], ALU.mult, ALU.add)
            nc.vector.tensor_copy(xfm[:, do, ts(sh, 512)], t1[:])

    emit_ln(0)
    emit_ln(1)

    # ==== vocab-shard head ====
    # Warm-up groups run all sh=0 token-half matmuls before any sh=1 work,
    # so the PE has LN(sh=1)-independent work while that half's layernorm
    # apply is still draining on the vector engine.
    WG = 2
    for g in range(WG):
        hwt = hwp.tile([P, 3, DSUB, P], F16, tag="hw")
        nc.sync.dma_start(hwt[:], hw_d[g])
        for sh in range(2):
            for t in range(3):
                vt = 3 * g + t
                ps = pa.tile([P, 512], F32, tag="pa")
                for ko in range(DSUB):
                    nc.tensor.matmul(ps[:], hwt[:, t, ko, :],
                                     xfm[:, ko, ts(sh, 512)],
                                     start=(ko == 0), stop=(ko == DSUB - 1))
                ot = outp.tile([P, 512], F16, tag="out", bufs=3)
                nc.vector.tensor_copy(ot[:], ps[:])
                eng = (nc.scalar, nc.gpsimd, nc.sync)[vt % 3]
                eng.dma_start(out_d[ts(vt, P), ts(sh, 512)], ot[:])
    for g in range(WG, VTS // 3):
        hwt = hwp.tile([P, 3, DSUB, P], F16, tag="hw")
        nc.sync.dma_start(hwt[:], hw_d[g])
        for t in range(3):
            vt = 3 * g + t
            ot = outp.tile([P, S], F16, tag="out", bufs=3)
            for sh in range(2):
                ps = pa.tile([P, 512], F32, tag="pa")
                for ko in range(DSUB):
                    nc.tensor.matmul(ps[:], hwt[:, t, ko, :],
                                     xfm[:, ko, ts(sh, 512)],
                                     start=(ko == 0), stop=(ko == DSUB - 1))
                nc.vector.tensor_copy(ot[:, ts(sh, 512)], ps[:])
            eng = (nc.scalar, nc.gpsimd, nc.sync)[vt % 3]
            eng.dma_start(out_d[ts(vt, P), :], ot[:])

    ctx.close()


_CACHE = {}


def _get_program():
    if "nc" not in _CACHE:
        _CACHE["nc"] = build_program()
    return _CACHE["nc"]


def make_in_maps(tokens, emb, Wq, Wk, Wv, Wb, Wo, rms_w, ln_g, ln_b, head_w):
    def arrange_w(w):  # [D, N] -> [128, DSUB, N] with (p, ko) striping of D
        return np.ascontiguousarray(
            w.astype(F16_NP).reshape(DSUB, P, -1).transpose(1, 0, 2))

    wq_h = np.stack([arrange_w(Wq[l]) for l in range(L)])
    wk_h = np.stack([arrange_w(Wk[l]) for l in range(L)])
    wv_h = np.stack([arrange_w(Wv[l]) for l in range(L)])
    wb_h = np.stack([arrange_w(np.repeat(Wb[l], 2, axis=1)) for l in range(L)])
    wo_h = np.stack([arrange_w(rms_w[l][:, None] * Wo[l]) for l in range(L)])
    emb_h = emb.astype(F16_NP)
    lng_h = np.ascontiguousarray(ln_g.reshape(DSUB, P).T)
    lnb_h = np.ascontiguousarray(ln_b.reshape(DSUB, P).T)

    in_maps = []
    for core in range(8):
        b, vs = core // 4, core % 4
        hw_pad = np.zeros((D, VSP), np.float32)
        hw_pad[:, :VS] = head_w[:, ts(vs, VS)]
        hw_h = np.ascontiguousarray(
            hw_pad.astype(F16_NP).reshape(DSUB, P, VTS // 3, 3, P)
            .transpose(2, 1, 3, 0, 4))
        tok_h = np.ascontiguousarray(
            tokens[b].astype(np.int32).reshape(NCH, P).T)
        in_maps.append({
            "tokens": tok_h, "emb": emb_h,
            "wq": wq_h, "wk": wk_h, "wv": wv_h, "wb": wb_h, "wo": wo_h,
            "lng": lng_h, "lnb": lnb_h, "hw": hw_h,
        })
    return in_maps


def assemble_output(results):
    out = np.empty((2, S, V), np.float32)
    for core in range(8):
        b, vs = core // 4, core % 4
        lt = results[core]["logits_t"]          # [VSP, S] bf16
        out[b, :, ts(vs, VS)] = np.ascontiguousarray(
            lt[:VS].astype(np.float32)).T
    return out


def kernel(tokens, emb, Wq, Wk, Wv, Wb, Wo, rms_w, ln_g, ln_b, head_w):
    tokens = np.asarray(tokens)
    args = [np.asarray(a, np.float32) for a in
            (emb, Wq, Wk, Wv, Wb, Wo, rms_w, ln_g, ln_b, head_w)]
    nc = _get_program()
    in_maps = make_in_maps(tokens, *args)
    res = run_bass_kernel_spmd(nc, in_maps, core_ids=list(range(8)),
                               trace=bool(_CACHE.get("trace")))
    _CACHE["last_result"] = res
    return assemble_output(res.results)
